# revision 76
# baseline (speedup 1.0000x reference)
"""Trainium2 Bass kernel for BandProcessorWithHistory (v2, algebraic rewrite).

Reference computation (full inputs):
    xn = LN(x, g1, be1); Q = xn@Wq.T + bq
    K = history@Wk.T + bk; V = history@Wv.T + bv          # [T,H,D]
    scores = einsum('btd,thd->bth', Q, K)/sqrt(D) + log(decay + 1e-10)
    attn = softmax(scores, -1); attended = einsum('bth,thd->btd', attn, V)
    x2 = x + attended@Wo.T + bo
    out = x2 + gelu(LN(x2,g2,be2)@W1.T + b1)@W2.T + b2

Key algebraic rewrite (removes the dominant K/V projections, 8x less work):
    scores  = Q.(H Wk^T)^T = (xn @ (Wq^T Wk)/sqrt(D)) . H^T        (+ bq@Wk;
              bk drops: softmax is invariant to a per-row constant)
    attended@Wo^T = (attn @ H) @ (Wo Wv)^T + Wo@bv   (rows of attn sum to 1)
so only [2048 x 512 x 512]-shaped projections remain per core, contracting
with raw history directly.  The per-position decay bias + block-diagonal
validity mask are folded into one additive f32 constant added to the scores
PSUM before exp; exp's accum_out produces softmax denominators for free.

Sharding: T (sequence) axis split over 8 cores (256 positions each);
embarrassingly parallel.  Activations feature-major [d on partitions, rows
on free dim], rows r = t_local*B + b.

Precision: matmuls run fp8e4(e4m3) with DoubleRow perf mode (2 k-chunks of
128 packed per instruction, ~1.5-2x PE throughput); weight matrices are
pre-scaled on the host into the fp8 normal range and descaled via activation
/vector-op scale constants.  LN statistics come from ones-matmuls (fp8 for
LN1 on the host-quantized x; exact f32r for LN2 on the on-chip x2).  The
per-row LN mean removal is algebraic: a k=1 rank-1 f32r matmul accumulated
into the downstream projection PSUM (-cq (x) mu for Q2, -(W1@g2) (x) mu*rs
for FFN1), with g2/be2 folded into W1/b1 on the host, so the LN apply is a
single broadcast-multiply.  Residual x rides in bf16 with bo' pre-added on
host; output is bf16.

All stages are software-pipelined 2-4 blocks deep so every engine queue
(in-order!) always holds ready work; attention transposes run on the PE in
fp8 (keeping the HAM clock-gate warm) with strided (step-2) PSUM outputs.
"""

import math
import os
from contextlib import ExitStack

import numpy as np
import ml_dtypes

import concourse.bacc as bacc
import concourse.bass as bass
import concourse.mybir as mybir
import concourse.tile as tile
from concourse.bass_utils import run_bass_kernel_spmd

F32 = mybir.dt.float32
F32R = mybir.dt.float32r
BF16 = mybir.dt.bfloat16
FP8 = mybir.dt.float8e4
DR = mybir.MatmulPerfMode.DoubleRow

B, T, H, D = 8, 2048, 64, 512
N_CORES = 8
T_LOC = T // N_CORES          # 256 positions per core
R = B * T_LOC                 # 2048 activation rows per core (r = t*B + b)
HR = T_LOC * H                # 16384 history rows per core
P = 128
DC = D // P                   # 4 chunks of the model dim
D2 = 2 * D
D2C = D2 // P                 # 8 chunks
BLK_T = 16                    # positions per attention block
N_BLK = T_LOC // BLK_T        # 16 blocks
HCOL = BLK_T * H              # 1024 history cols per block
RB = 512                      # r-columns per projection block
N_RB = R // RB                # 4
DECAY_RATE = 0.95
LN_EPS = 1e-5

# fp8 weight pre-scales (descaled on-chip via activation/vector constants)
S_A = 4096.0                  # LN1-folded Wq^T.Wk weight
S_Q = 256.0                   # Q2 activation scale (descaled inside exp)
S_O = 512.0                   # Wo.Wv weight
S_1 = 128.0                   # W1
S_2 = 128.0                   # W2
NEG_BIG = -30.0               # additive mask for invalid score entries

_last_result = [None]
_cached = {}

AF = mybir.ActivationFunctionType
OP = mybir.AluOpType


def _build_program():
    nc = bacc.Bacc("TRN2", target_bir_lowering=False, debug=False)

    xq8d = nc.dram_tensor("xq8", [P, DC, R], FP8, kind="ExternalInput")
    xb16d = nc.dram_tensor("xb16", [P, DC, R], BF16, kind="ExternalInput")
    hfmd = nc.dram_tensor("hfm8", [P, DC, HR], FP8, kind="ExternalInput")
    hrmd = nc.dram_tensor("hrm8", [P, HR // P, D], FP8, kind="ExternalInput")
    wad = nc.dram_tensor("wa8", [P, DC, D], FP8, kind="ExternalInput")
    wovd = nc.dram_tensor("wov8", [P, DC, D], FP8, kind="ExternalInput")
    w1d = nc.dram_tensor("w18", [P, DC, D2], FP8, kind="ExternalInput")
    w2d = nc.dram_tensor("w28", [P, D2C, D], FP8, kind="ExternalInput")
    maskd = nc.dram_tensor("maskS", [P, HCOL], F32, kind="ExternalInput")
    id8d = nc.dram_tensor("ident8", [P, P], FP8, kind="ExternalInput")
    dqd = nc.dram_tensor("dq", [P, DC], F32, kind="ExternalInput")
    b1cd = nc.dram_tensor("b1c", [P, D2C], F32, kind="ExternalInput")
    b2cd = nc.dram_tensor("b2c", [P, DC], F32, kind="ExternalInput")
    be2d = nc.dram_tensor("be2c", [P, DC], F32, kind="ExternalInput")
    onesAd = nc.dram_tensor("onesA", [1, P], F32R, kind="ExternalInput")
    cqnd = nc.dram_tensor("cqn", [1, D], F32R, kind="ExternalInput")
    ones1d = nc.dram_tensor("ones1", [1, P], F32R, kind="ExternalInput")
    cf1nd = nc.dram_tensor("cf1n", [1, D2], F32R, kind="ExternalInput")
    ones8d = nc.dram_tensor("ones8", [P, 2, 16], FP8, kind="ExternalInput")
    onesrd = nc.dram_tensor("onesr", [P, 1], F32R, kind="ExternalInput")
    outd = nc.dram_tensor("outT", [P, DC, R], BF16, kind="ExternalOutput")

    with tile.TileContext(nc) as tc, ExitStack() as top:
        const = top.enter_context(tc.tile_pool(name="const", bufs=1))
        pers = top.enter_context(tc.tile_pool(name="pers", bufs=1))

        # ---- constants resident for the whole kernel ----
        wa_t = const.tile([P, DC, D], FP8)
        ones8_t = const.tile([P, 2, 16], FP8)
        eps1 = const.tile([1, 1], F32)
        nc.vector.memset(eps1[:], LN_EPS)
        onesA_t = const.tile([1, P], F32R)
        cqn_t = const.tile([1, D], F32R)
        dq_t = const.tile([P, DC], F32)
        mask_t = const.tile([P, HCOL], F32)
        id8_t = const.tile([P, P], FP8)
        wov_t = const.tile([P, DC, D], FP8)
        w1_t = const.tile([P, DC, D2], FP8)
        w2_t = const.tile([P, D2C, D], FP8)
        ones1_t = const.tile([1, P], F32R)
        cf1n_t = const.tile([1, D2], F32R)
        b1c_t = const.tile([P, D2C], F32)
        b2c_t = const.tile([P, DC], F32)

        nc.sync.dma_start(wa_t[:], wad[:])
        nc.sync.dma_start(ones8_t[:], ones8d[:])
        nc.sync.dma_start(onesA_t[:], onesAd[:])
        nc.sync.dma_start(cqn_t[:], cqnd[:])
        nc.sync.dma_start(dq_t[:], dqd[:])

        def load_late_consts():
            nc.sync.dma_start(xb16_t[:], xb16d[:])
            nc.sync.dma_start(mask_t[:], maskd[:])
            nc.sync.dma_start(id8_t[:], id8d[:])
            nc.sync.dma_start(wov_t[:], wovd[:])
            nc.sync.dma_start(w1_t[:], w1d[:])
            nc.sync.dma_start(w2_t[:], w2d[:])
            nc.sync.dma_start(ones1_t[:], ones1d[:])
            nc.sync.dma_start(cf1n_t[:], cf1nd[:])
            nc.sync.dma_start(b1c_t[:], b1cd[:])
            nc.sync.dma_start(b2c_t[:], b2cd[:])

        # ---- per-core resident activations ----
        xq8_t = pers.tile([P, DC, R], FP8)
        xb16_t = pers.tile([P, DC, R], BF16)
        q2_t = pers.tile([P, DC, R], FP8)      # scaled by S_Q
        att8_t = pers.tile([P, DC, R], FP8)    # attended, feature-major



        def ln_stats(pool, stats_ps, src8, sq8, tag, bufs=2, rs_bufs=3):
            """LN stats of one [128, DC, RB] fp8 chunk -> (rs, sh) f32r rows.

            src8: fp8 source; sq8: same-shape scratch for squares.
            """
            with nc.allow_low_precision(reason="fp8 square"):
                nc.scalar.square(sq8[:], src8)
            ps = stats_ps.tile([16, 2, RB], F32, tag="st", bufs=bufs)
            for pr in range(2):
                nc.tensor.matmul(ps[:, 0], ones8_t[:],
                                 src8[:, 2 * pr : 2 * pr + 2],
                                 start=pr == 0, stop=pr == 1, perf_mode=DR)
            for pr in range(2):
                nc.tensor.matmul(ps[:, 1], ones8_t[:],
                                 sq8[:, 2 * pr : 2 * pr + 2],
                                 start=pr == 0, stop=pr == 1, perf_mode=DR)
            st = pool.tile([1, 3, RB], F32, tag=f"st{tag}", bufs=3)
            mu, m2, var = st[:, 0], st[:, 1], st[:, 2]
            nc.vector.tensor_scalar(mu, ps[0:1, 0], 1.0 / D, None, OP.mult)
            nc.gpsimd.tensor_tensor(m2, mu, mu, OP.mult)
            nc.vector.scalar_tensor_tensor(var, ps[0:1, 1], 1.0 / D, m2,
                                           OP.mult, OP.subtract)
            std = st[:, 1]  # reuse m2 slot
            nc.scalar.activation(std, var, AF.Sqrt, bias=eps1[:])
            rsf = st[:, 2]  # reuse var slot
            nc.vector.reciprocal_approx_fast(rsf, std)
            rssh = pool.tile([1, 2, RB], F32R, tag=f"rs{tag}", bufs=rs_bufs)
            with nc.allow_low_precision(reason="f32r matmul operand"):
                nc.gpsimd.tensor_copy(rssh[:, 0], rsf)
                # row 1: plain mean (the rank-1 mean-removal matmuls fold
                # the rs factor via the rs broadcast that follows)
                nc.gpsimd.tensor_copy(rssh[:, 1], mu)
            return rssh

        # attention SBUF pool + history loads live across stage A so the
        # first blocks' DMA happens under the projection phase
        hstack = ExitStack()
        hpool = hstack.enter_context(tc.tile_pool(name="attn_sb", bufs=1))
        st1 = {}  # blk -> (hf, hrt)

        def s_load(blk):
            hf = hpool.tile([P, DC, HCOL], FP8, tag="hf", bufs=7)
            nc.sync.dma_start(hf[:], hfmd[:, :, blk * HCOL :
                                            (blk + 1) * HCOL])
            hrt = hpool.tile([P, D2C, D], FP8, tag="hr", bufs=8)
            nc.sync.dma_start(hrt[:],
                              hrmd[:, blk * D2C : (blk + 1) * D2C, :])
            st1[blk] = (hf, hrt)

        # ================= Stage A: LN1 + Q2 projection =================
        with ExitStack() as ctx:
            apool = ctx.enter_context(tc.tile_pool(name="stage_a", bufs=2))
            stats_ps = ctx.enter_context(
                tc.tile_pool(name="a_stats", bufs=2, space="PSUM"))
            bc_ps = ctx.enter_context(
                tc.tile_pool(name="a_bcast", bufs=2, space="PSUM"))
            mm_ps = ctx.enter_context(
                tc.tile_pool(name="a_mm", bufs=2, space="PSUM"))

            astate = {}

            def a_front(rb):
                rsl = slice(rb * RB, (rb + 1) * RB)
                nc.sync.dma_start(xq8_t[:, :, rsl], xq8d[:, :, rsl])
                sq8 = apool.tile([P, DC, RB], FP8, tag="sq", bufs=2)
                astate[rb] = ln_stats(apool, stats_ps, xq8_t[:, :, rsl],
                                      sq8, "a", rs_bufs=4)

            def a_back(rb):
                rsl = slice(rb * RB, (rb + 1) * RB)
                rssh = astate.pop(rb)
                ps_rsb = bc_ps.tile([P, RB], F32, tag="bc", bufs=2)
                nc.tensor.matmul(ps_rsb[:], onesA_t[:], rssh[:, 0],
                                 start=True, stop=True)
                rsb = apool.tile([P, RB], F32, tag="rsb", bufs=2)
                nc.scalar.copy(rsb[:], ps_rsb[:])
                for oc in range(DC):
                    ps_y = mm_ps.tile([P, RB], F32, tag="mm", bufs=2)
                    for pr in range(2):
                        nc.tensor.matmul(
                            ps_y[:], wa_t[:, 2 * pr : 2 * pr + 2,
                                          oc * P : (oc + 1) * P],
                            xq8_t[:, 2 * pr : 2 * pr + 2, rsl],
                            start=pr == 0, stop=False, perf_mode=DR)
                    # mean removal as a rank-1 accumulation: -S_A*cq (x) mu
                    nc.tensor.matmul(ps_y[:],
                                     cqn_t[:, oc * P : (oc + 1) * P],
                                     rssh[:, 1], start=False, stop=True)
                    t = apool.tile([P, RB], F32, tag="t", bufs=2)
                    nc.vector.tensor_tensor(t[:], ps_y[:], rsb[:], OP.mult)
                    with nc.allow_low_precision(reason="fp8 activation"):
                        nc.scalar.activation(q2_t[:, oc, rsl], t[:],
                                             AF.Identity,
                                             bias=dq_t[:, oc : oc + 1])

            a_front(0)
            a_front(1)
            load_late_consts()
            s_load(0)
            a_front(2)
            s_load(1)
            a_front(3)
            s_load(2)
            for rb in range(N_RB):
                a_back(rb)
                s_load(3 + rb)

        # ================= Stage B/C: attention =================
        # Software-pipelined; all transposes run on the PE in fp8 so the
        # tensor engine stays dense (HAM warm) and no DMA xbar is needed.
        # The decay/validity mask is a k=17 fp16 matmul accumulated into the
        # scores PSUM; exp's bias adds the -30 invalid offset and accum_out
        # yields softmax denominators.
        with ExitStack() as ctx:
            sc_ps = ctx.enter_context(
                tc.tile_pool(name="scores", bufs=2, space="PSUM"))
            ah_ps = ctx.enter_context(
                tc.tile_pool(name="attend", bufs=2, space="PSUM"))
            tr_ps = ctx.enter_context(
                tc.tile_pool(name="attn_tr", bufs=2, space="PSUM"))

            st2 = {}  # blk -> (am, den)
            st3 = {}  # blk -> amT8
            st4 = {}  # blk -> (atb8, r0)

            def s_scores(blk):
                hf, _ = st1[blk]
                r0 = blk * P
                ps_sc = sc_ps.tile([P, HCOL], F32, tag="sc", bufs=2)
                for nb in range(2):
                    for pr in range(2):
                        nc.tensor.matmul(
                            ps_sc[:, nb * RB : (nb + 1) * RB],
                            q2_t[:, 2 * pr : 2 * pr + 2, r0 : r0 + P],
                            hf[:, 2 * pr : 2 * pr + 2, nb * RB : (nb + 1) * RB],
                            start=pr == 0, stop=pr == 1, perf_mode=DR)
                nc.vector.tensor_tensor(ps_sc[:], ps_sc[:], mask_t[:], OP.add)
                am = hpool.tile([P, HCOL], FP8, tag="am", bufs=4)
                den = hpool.tile([P, 2], F32, tag="den", bufs=5)
                with nc.allow_low_precision(reason="fp8 attn probs"):
                    nc.scalar.activation(am[:], ps_sc[:], AF.Exp,
                                         scale=1.0 / S_Q,
                                         accum_out=den[:, 0:1])
                nc.vector.reciprocal_approx_fast(den[:, 1:2], den[:, 0:1])
                st2[blk] = (am, den)

            def s_trans(blk):
                am, _ = st2[blk]
                # fp8 PE-transpose writes with an element step of 2 in PSUM
                ps_t = tr_ps.tile([P, D2C, P, 2], FP8, tag="tr", bufs=2)
                for ch in range(D2C):
                    nc.tensor.transpose(ps_t[:, ch, :, 0],
                                        am[:, ch * P : (ch + 1) * P], id8_t[:])
                amT8 = hpool.tile([P, D2C, P], FP8, tag="amT8", bufs=3)
                with nc.allow_low_precision(reason="fp8 attn probs"):
                    nc.scalar.copy(amT8[:], ps_t[:, :, :, 0])
                st3[blk] = amT8

            def s_attend(blk):
                _, hrt = st1.pop(blk)
                _, den = st2.pop(blk)
                amT8 = st3.pop(blk)
                ps_ah = ah_ps.tile([P, D], F32, tag="ah", bufs=2)
                for pr in range(4):
                    nc.tensor.matmul(ps_ah[:], amT8[:, 2 * pr : 2 * pr + 2],
                                     hrt[:, 2 * pr : 2 * pr + 2],
                                     start=pr == 0, stop=pr == 3, perf_mode=DR)
                atb8 = hpool.tile([P, D], FP8, tag="atb8", bufs=3)
                with nc.allow_low_precision(reason="fp8 attended"):
                    nc.vector.tensor_scalar_mul(atb8[:], ps_ah[:], den[:, 1:2])
                st4[blk] = atb8

            def s_post(blk):
                atb8 = st4.pop(blk)
                r0 = blk * P
                ps_t = tr_ps.tile([P, D2C, P, 2], FP8, tag="tr", bufs=2)
                for ec in range(DC):
                    nc.tensor.transpose(ps_t[:, ec, :, 0],
                                        atb8[:, ec * P : (ec + 1) * P],
                                        id8_t[:])
                nc.vector.tensor_copy(att8_t[:, :, r0 : r0 + P],
                                      ps_t[:, :DC, :, 0])

            for i in range(N_BLK + 4):
                if 7 <= i + 2 < N_BLK:
                    s_load(i + 2)
                if 2 <= i < N_BLK + 2:
                    s_trans(i - 2)
                if 3 <= i < N_BLK + 3:
                    s_attend(i - 3)
                if i >= 4:
                    s_post(i - 4)
                if i < N_BLK:
                    s_scores(i)

        hstack.close()

        # ================= Stage D/E: Wov + LN2 + FFN =================
        # Three pipeline stages per rb: D (Wov projection + residual + LN2
        # stats, exact f32), E1 (LN2 apply + FFN1/gelu), E2 (FFN2 + out).
        with ExitStack() as ctx:
            epool = ctx.enter_context(tc.tile_pool(name="stage_e", bufs=2))
            stats_ps = ctx.enter_context(
                tc.tile_pool(name="e_stats", bufs=1, space="PSUM"))
            bc_ps = ctx.enter_context(
                tc.tile_pool(name="e_bcast", bufs=1, space="PSUM"))
            mm_ps = ctx.enter_context(
                tc.tile_pool(name="e_mm", bufs=3, space="PSUM"))

            onesr = const.tile([P, 1], F32R)
            nc.sync.dma_start(onesr[:], onesrd[:])

            dstate = {}
            estate = {}
            fstate = {}

            def e_projd(rb):
                rsl = slice(rb * RB, (rb + 1) * RB)
                x2 = epool.tile([P, DC, RB], F32R, tag="x2", bufs=4)
                for oc in range(DC):
                    ps_w = mm_ps.tile([P, RB], F32, tag="mm", bufs=3)
                    for pr in range(2):
                        nc.tensor.matmul(
                            ps_w[:], wov_t[:, 2 * pr : 2 * pr + 2,
                                           oc * P : (oc + 1) * P],
                            att8_t[:, 2 * pr : 2 * pr + 2, rsl],
                            start=pr == 0, stop=pr == 1, perf_mode=DR)
                    with nc.allow_low_precision(reason="f32r residual"):
                        nc.vector.scalar_tensor_tensor(
                            x2[:, oc], ps_w[:], 1.0 / S_O, xb16_t[:, oc, rsl],
                            OP.mult, OP.add)
                # exact f32 LN stats via f32r ones-matmuls on x2
                sqf = epool.tile([P, DC, RB], F32R, tag="sqf", bufs=2)
                with nc.allow_low_precision(reason="f32r square"):
                    nc.scalar.square(sqf[:], x2[:])
                ps = stats_ps.tile([1, 2, RB], F32, tag="st", bufs=1)
                for dc in range(DC):
                    nc.tensor.matmul(ps[:, 0], onesr[:], x2[:, dc],
                                     start=dc == 0, stop=dc == DC - 1)
                for dc in range(DC):
                    nc.tensor.matmul(ps[:, 1], onesr[:], sqf[:, dc],
                                     start=dc == 0, stop=dc == DC - 1)
                st = epool.tile([1, 3, RB], F32, tag="ste", bufs=2)
                mu, m2, var = st[:, 0], st[:, 1], st[:, 2]
                nc.vector.tensor_copy(mu, ps[0:1, 0])
                nc.gpsimd.tensor_tensor(m2, mu, mu, OP.mult)
                nc.vector.tensor_tensor(var, ps[0:1, 1], m2, OP.subtract)
                std = st[:, 1]
                nc.scalar.activation(std, var, AF.Sqrt, bias=eps1[:])
                rsf = st[:, 2]
                nc.vector.reciprocal_approx_fast(rsf, std)
                rssh = epool.tile([1, 2, RB], F32R, tag="rse", bufs=3)
                with nc.allow_low_precision(reason="f32r matmul operand"):
                    nc.gpsimd.tensor_copy(rssh[:, 0], rsf)
                    nc.gpsimd.tensor_tensor(rssh[:, 1], mu, rsf, OP.mult)
                dstate[rb] = (x2, rssh)

            def e_ln2(rb):
                x2, rssh = dstate.pop(rb)
                # h2_hat = x2 * rs  (g2 folded into W1; mean term removed
                # inside FFN1's PSUM as a rank-1 matmul; be2 folded into b1)
                ps_rs = bc_ps.tile([P, RB], F32, tag="bc", bufs=2)
                nc.tensor.matmul(ps_rs[:], ones1_t[:], rssh[:, 0],
                                 start=True, stop=True)
                h28 = epool.tile([P, DC, RB], FP8, tag="h2", bufs=2)
                with nc.allow_low_precision(reason="fp8 LN2 out"):
                    for dc in range(DC):
                        nc.vector.tensor_tensor(h28[:, dc], x2[:, dc],
                                                ps_rs[:], OP.mult)
                estate[rb] = (x2, rssh, h28)

            def e_ffn1(rb):
                x2, rssh, h28 = estate.pop(rb)
                a18 = epool.tile([P, D2C, RB], FP8, tag="a1", bufs=2)
                for oc in range(D2C):
                    ps_f = mm_ps.tile([P, RB], F32, tag="mm", bufs=3)
                    for pr in range(2):
                        nc.tensor.matmul(
                            ps_f[:], w1_t[:, 2 * pr : 2 * pr + 2,
                                          oc * P : (oc + 1) * P],
                            h28[:, 2 * pr : 2 * pr + 2],
                            start=pr == 0, stop=False, perf_mode=DR)
                    # rank-1 mean removal: -S_1*(W1@g2) (x) (mu*rs)
                    nc.tensor.matmul(ps_f[:],
                                     cf1n_t[:, oc * P : (oc + 1) * P],
                                     rssh[:, 1], start=False, stop=True)
                    with nc.allow_low_precision(reason="fp8 gelu"):
                        nc.scalar.activation(a18[:, oc], ps_f[:], AF.Gelu,
                                             bias=b1c_t[:, oc : oc + 1],
                                             scale=1.0 / S_1)
                fstate[rb] = (x2, a18)

            def e_ffn2(rb):
                rsl = slice(rb * RB, (rb + 1) * RB)
                x2, a18 = fstate.pop(rb)
                ot = epool.tile([P, DC, RB], BF16, tag="ot", bufs=2)
                tf = epool.tile([P, RB], F32, tag="tf", bufs=3)
                for oc in range(DC):
                    ps_f = mm_ps.tile([P, RB], F32, tag="mm", bufs=3)
                    for pr in range(4):
                        nc.tensor.matmul(
                            ps_f[:], w2_t[:, 2 * pr : 2 * pr + 2,
                                          oc * P : (oc + 1) * P],
                            a18[:, 2 * pr : 2 * pr + 2],
                            start=pr == 0, stop=pr == 3, perf_mode=DR)
                    nc.scalar.activation(tf[:], ps_f[:], AF.Identity,
                                         bias=b2c_t[:, oc : oc + 1],
                                         scale=1.0 / S_2)
                    with nc.allow_low_precision(reason="bf16 output"):
                        nc.vector.tensor_tensor(ot[:, oc], tf[:], x2[:, oc],
                                                OP.add)
                nc.sync.dma_start(outd[:, :, rsl], ot[:])

            e_projd(0)
            e_projd(1)
            e_ln2(0)
            e_projd(2)
            e_ffn1(0)
            e_ln2(1)
            e_projd(3)
            e_ffn2(0)
            e_ffn1(1)
            e_ln2(2)
            e_ffn2(1)
            e_ffn1(2)
            e_ln2(3)
            e_ffn2(2)
            e_ffn1(3)
            e_ffn2(3)

    nc.compile()
    return nc


def _tile_fm(a, dt):
    """[Dred, N] feature-major -> [128, Dred//128, N] device tiling."""
    dred, n = a.shape
    return np.ascontiguousarray(
        a.reshape(dred // P, P, n).swapaxes(0, 1)).astype(dt)


def _vec_pc(v):
    """[n*128] vector -> [128, n] (partition, chunk) f32."""
    return np.ascontiguousarray(np.asarray(v, np.float64).reshape(-1, P).T
                                ).astype(np.float32)


def _make_weight_map(inputs):
    f64 = {k: np.asarray(v, np.float64) for k, v in inputs.items()}
    isd = 1.0 / math.sqrt(D)

    WQK = (f64["Wq"].T @ f64["Wk"]) * isd          # [f, e]
    WA = f64["g1"][:, None] * WQK
    cq = WA.sum(axis=0)                            # [e]
    dq = f64["be1"] @ WQK + f64["bq"] @ f64["Wk"] * isd
    WOV = f64["Wo"] @ f64["Wv"]                    # [d, f]
    bo_p = f64["bo"] + f64["Wo"] @ f64["bv"]

    wa8 = _tile_fm(WA * S_A, ml_dtypes.float8_e4m3)
    wov8 = _tile_fm(WOV.T * S_O, ml_dtypes.float8_e4m3)
    W1g = f64["W1"] * f64["g2"][None, :]          # g2 folded into W1
    w18 = _tile_fm(W1g.T * S_1, ml_dtypes.float8_e4m3)
    w28 = _tile_fm(f64["W2"].T * S_2, ml_dtypes.float8_e4m3)
    cf1n = (-S_1 * (f64["W1"] @ f64["g2"]))[None, :]
    b1f = f64["b1"] + f64["W1"] @ f64["be2"]      # be2 folded into b1

    # additive scores mask: S_Q*log(decay) on valid (t-matching) entries,
    # S_Q*(-30) on invalid ones (exp flushes those to 0)
    h = np.arange(H)
    logdecay = np.log(DECAY_RATE ** (H - 1 - h) + 1e-10)
    maskS = np.full((P, HCOL), NEG_BIG * S_Q, np.float64)
    for p_ in range(P):
        t = p_ // B
        maskS[p_, t * H : (t + 1) * H] = S_Q * logdecay
    ones8 = np.ones((P, 2, 16), ml_dtypes.float8_e4m3)

    return dict(
        wa8=wa8, wov8=wov8, w18=w18, w28=w28,
        maskS=maskS.astype(np.float32),
        ident8=np.eye(P, dtype=ml_dtypes.float8_e4m3),
        dq=_vec_pc(dq * S_Q),
        b1c=_vec_pc(b1f),
        b2c=_vec_pc(f64["b2"]),
        be2c=_vec_pc(f64["be2"]),
        onesA=np.full((1, P), S_Q / S_A, np.float32),
        cqn=(-cq * S_A)[None, :].astype(np.float32),
        ones1=np.full((1, P), 1.0, np.float32),
        cf1n=cf1n.astype(np.float32),
        ones8=ones8,
        onesr=np.full((P, 1), 1.0 / D, np.float32),
        _bo_p=bo_p,  # consumed by core_input_map, not a dram tensor
    )


def core_input_map(inputs, wmap, c):
    """Per-core input dict (core c owns positions [c*T_LOC, (c+1)*T_LOC))."""
    x = np.asarray(inputs["x"], np.float32)
    history = np.asarray(inputs["history"], np.float32)
    ts = slice(c * T_LOC, (c + 1) * T_LOC)
    xr = x[:, ts, :].transpose(1, 0, 2).reshape(R, D)      # r = t*B + b
    hr = history[ts].reshape(HR, D)
    m = {k: v for k, v in wmap.items() if not k.startswith("_")}
    m["xq8"] = _tile_fm(np.ascontiguousarray(xr.T), ml_dtypes.float8_e4m3)
    m["xb16"] = _tile_fm(np.ascontiguousarray((xr + wmap["_bo_p"]).T),
                         ml_dtypes.bfloat16)
    m["hfm8"] = _tile_fm(np.ascontiguousarray(hr.T), ml_dtypes.float8_e4m3)
    m["hrm8"] = np.ascontiguousarray(
        hr.reshape(HR // P, P, D).swapaxes(0, 1)).astype(ml_dtypes.float8_e4m3)
    return m


def unpack_out(ot):
    """[128, DC, R] bf16 device tile -> [B, T_LOC, D] f32."""
    full = np.asarray(ot, np.float32).swapaxes(0, 1).reshape(D, R).T
    return full.reshape(T_LOC, B, D).transpose(1, 0, 2)


def kernel(**inputs):
    if "nc" not in _cached:
        _cached["nc"] = _build_program()
    nc = _cached["nc"]

    wmap = _make_weight_map(inputs)
    in_maps = [core_input_map(inputs, wmap, c) for c in range(N_CORES)]

    res = run_bass_kernel_spmd(nc, in_maps, core_ids=list(range(N_CORES)))
    _last_result[0] = res

    out = np.empty((B, T, D), dtype=np.float32)
    for c in range(N_CORES):
        ts = slice(c * T_LOC, (c + 1) * T_LOC)
        out[:, ts, :] = unpack_out(res.results[c]["outT"])
    return out


# revision 80
# speedup vs baseline: 1.0226x; 1.0226x over previous
"""Trainium2 Bass kernel for BandProcessorWithHistory (v2, algebraic rewrite).

Reference computation (full inputs):
    xn = LN(x, g1, be1); Q = xn@Wq.T + bq
    K = history@Wk.T + bk; V = history@Wv.T + bv          # [T,H,D]
    scores = einsum('btd,thd->bth', Q, K)/sqrt(D) + log(decay + 1e-10)
    attn = softmax(scores, -1); attended = einsum('bth,thd->btd', attn, V)
    x2 = x + attended@Wo.T + bo
    out = x2 + gelu(LN(x2,g2,be2)@W1.T + b1)@W2.T + b2

Key algebraic rewrite (removes the dominant K/V projections, 8x less work):
    scores  = Q.(H Wk^T)^T = (xn @ (Wq^T Wk)/sqrt(D)) . H^T        (+ bq@Wk;
              bk drops: softmax is invariant to a per-row constant)
    attended@Wo^T = (attn @ H) @ (Wo Wv)^T + Wo@bv   (rows of attn sum to 1)
so only [2048 x 512 x 512]-shaped projections remain per core, contracting
with raw history directly.  The per-position decay bias + block-diagonal
validity mask are folded into one additive f32 constant added to the scores
PSUM before exp; exp's accum_out produces softmax denominators for free.

Sharding: T (sequence) axis split over 8 cores (256 positions each);
embarrassingly parallel.  Activations feature-major [d on partitions, rows
on free dim], rows r = t_local*B + b.

Precision: matmuls run fp8e4(e4m3) with DoubleRow perf mode (2 k-chunks of
128 packed per instruction, ~1.5-2x PE throughput); weight matrices are
pre-scaled on the host into the fp8 normal range and descaled via activation
/vector-op scale constants.  LN statistics come from ones-matmuls (fp8 for
LN1 on the host-quantized x; exact f32r for LN2 on the on-chip x2).  The
per-row LN mean removal is algebraic: a k=1 rank-1 f32r matmul accumulated
into the downstream projection PSUM (-cq (x) mu for Q2, -(W1@g2) (x) mu*rs
for FFN1), with g2/be2 folded into W1/b1 on the host, so the LN apply is a
single broadcast-multiply.  Residual x rides in bf16 with bo' pre-added on
host; output is bf16.

All stages are software-pipelined 2-4 blocks deep so every engine queue
(in-order!) always holds ready work; attention transposes run on the PE in
fp8 (keeping the HAM clock-gate warm) with strided (step-2) PSUM outputs.
"""

import math
import os
from contextlib import ExitStack

import numpy as np
import ml_dtypes

import concourse.bacc as bacc
import concourse.bass as bass
import concourse.mybir as mybir
import concourse.tile as tile
from concourse.bass_utils import run_bass_kernel_spmd

F32 = mybir.dt.float32
F32R = mybir.dt.float32r
BF16 = mybir.dt.bfloat16
FP8 = mybir.dt.float8e4
DR = mybir.MatmulPerfMode.DoubleRow

B, T, H, D = 8, 2048, 64, 512
N_CORES = 8
T_LOC = T // N_CORES          # 256 positions per core
R = B * T_LOC                 # 2048 activation rows per core (r = t*B + b)
HR = T_LOC * H                # 16384 history rows per core
P = 128
DC = D // P                   # 4 chunks of the model dim
D2 = 2 * D
D2C = D2 // P                 # 8 chunks
BLK_T = 16                    # positions per attention block
N_BLK = T_LOC // BLK_T        # 16 blocks
HCOL = BLK_T * H              # 1024 history cols per block
RB = 512                      # r-columns per projection block
N_RB = R // RB                # 4
DECAY_RATE = 0.95
LN_EPS = 1e-5

# fp8 weight pre-scales (descaled on-chip via activation/vector constants)
S_A = 4096.0                  # LN1-folded Wq^T.Wk weight
S_Q = 256.0                   # Q2 activation scale (descaled inside exp)
S_O = 512.0                   # Wo.Wv weight
S_1 = 128.0                   # W1
S_2 = 128.0                   # W2
NEG_BIG = -30.0               # additive mask for invalid score entries

_last_result = [None]
_cached = {}

AF = mybir.ActivationFunctionType
OP = mybir.AluOpType


def _build_program():
    nc = bacc.Bacc("TRN2", target_bir_lowering=False, debug=False)

    xq8d = nc.dram_tensor("xq8", [P, DC, R], FP8, kind="ExternalInput")
    xb16d = nc.dram_tensor("xb16", [P, DC, R], BF16, kind="ExternalInput")
    hfmd = nc.dram_tensor("hfm8", [P, DC, HR], FP8, kind="ExternalInput")
    hrmd = nc.dram_tensor("hrm8", [P, HR // P, D], FP8, kind="ExternalInput")
    wad = nc.dram_tensor("wa8", [P, DC, D], FP8, kind="ExternalInput")
    wovd = nc.dram_tensor("wov8", [P, DC, D], FP8, kind="ExternalInput")
    w1d = nc.dram_tensor("w18", [P, DC, D2], FP8, kind="ExternalInput")
    w2d = nc.dram_tensor("w28", [P, D2C, D], FP8, kind="ExternalInput")
    maskd = nc.dram_tensor("maskS", [P, HCOL], F32, kind="ExternalInput")
    id8d = nc.dram_tensor("ident8", [P, P], FP8, kind="ExternalInput")
    dqd = nc.dram_tensor("dq", [P, DC], F32, kind="ExternalInput")
    b1cd = nc.dram_tensor("b1c", [P, D2C], F32, kind="ExternalInput")
    b2cd = nc.dram_tensor("b2c", [P, DC], F32, kind="ExternalInput")
    be2d = nc.dram_tensor("be2c", [P, DC], F32, kind="ExternalInput")
    onesAd = nc.dram_tensor("onesA", [1, P], F32R, kind="ExternalInput")
    cqnd = nc.dram_tensor("cqn", [1, D], F32R, kind="ExternalInput")
    ones1d = nc.dram_tensor("ones1", [1, P], F32R, kind="ExternalInput")
    cf1nd = nc.dram_tensor("cf1n", [1, D2], F32R, kind="ExternalInput")
    ones8d = nc.dram_tensor("ones8", [P, 2, 16], FP8, kind="ExternalInput")
    onesrd = nc.dram_tensor("onesr", [P, 1], F32R, kind="ExternalInput")
    outd = nc.dram_tensor("outT", [P, DC, R], BF16, kind="ExternalOutput")

    with tile.TileContext(nc) as tc, ExitStack() as top:
        const = top.enter_context(tc.tile_pool(name="const", bufs=1))
        pers = top.enter_context(tc.tile_pool(name="pers", bufs=1))

        # ---- constants resident for the whole kernel ----
        wa_t = const.tile([P, DC, D], FP8)
        ones8_t = const.tile([P, 2, 16], FP8)
        eps1 = const.tile([1, 1], F32)
        nc.vector.memset(eps1[:], LN_EPS)
        onesA_t = const.tile([1, P], F32R)
        cqn_t = const.tile([1, D], F32R)
        dq_t = const.tile([P, DC], F32)
        mask_t = const.tile([P, HCOL], F32)
        id8_t = const.tile([P, P], FP8)
        wov_t = const.tile([P, DC, D], FP8)
        w1_t = const.tile([P, DC, D2], FP8)
        w2_t = const.tile([P, D2C, D], FP8)
        ones1_t = const.tile([1, P], F32R)
        cf1n_t = const.tile([1, D2], F32R)
        b1c_t = const.tile([P, D2C], F32)
        b2c_t = const.tile([P, DC], F32)

        nc.sync.dma_start(wa_t[:], wad[:])
        nc.sync.dma_start(ones8_t[:], ones8d[:])
        nc.sync.dma_start(onesA_t[:], onesAd[:])
        nc.sync.dma_start(cqn_t[:], cqnd[:])
        nc.sync.dma_start(dq_t[:], dqd[:])

        def load_late_consts():
            nc.sync.dma_start(xb16_t[:], xb16d[:])
            nc.sync.dma_start(mask_t[:], maskd[:])
            nc.sync.dma_start(id8_t[:], id8d[:])
            nc.sync.dma_start(wov_t[:], wovd[:])
            nc.sync.dma_start(w1_t[:], w1d[:])
            nc.sync.dma_start(w2_t[:], w2d[:])
            nc.sync.dma_start(ones1_t[:], ones1d[:])
            nc.sync.dma_start(cf1n_t[:], cf1nd[:])
            nc.sync.dma_start(b1c_t[:], b1cd[:])
            nc.sync.dma_start(b2c_t[:], b2cd[:])

        # ---- per-core resident activations ----
        xq8_t = pers.tile([P, DC, R], FP8)
        xb16_t = pers.tile([P, DC, R], BF16)
        q2_t = pers.tile([P, DC, R], FP8)      # scaled by S_Q
        att8_t = pers.tile([P, DC, R], FP8)    # attended, feature-major



        def ln_stats(pool, stats_ps, src8, sq8, tag, bufs=2, rs_bufs=3):
            """LN stats of one [128, DC, RB] fp8 chunk -> (rs, sh) f32r rows.

            src8: fp8 source; sq8: same-shape scratch for squares.
            """
            with nc.allow_low_precision(reason="fp8 square"):
                nc.scalar.square(sq8[:], src8)
            ps = stats_ps.tile([16, 2, RB], F32, tag="st", bufs=bufs)
            for pr in range(2):
                nc.tensor.matmul(ps[:, 0], ones8_t[:],
                                 src8[:, 2 * pr : 2 * pr + 2],
                                 start=pr == 0, stop=pr == 1, perf_mode=DR)
            for pr in range(2):
                nc.tensor.matmul(ps[:, 1], ones8_t[:],
                                 sq8[:, 2 * pr : 2 * pr + 2],
                                 start=pr == 0, stop=pr == 1, perf_mode=DR)
            st = pool.tile([1, 3, RB], F32, tag=f"st{tag}", bufs=3)
            mu, m2, var = st[:, 0], st[:, 1], st[:, 2]
            nc.vector.tensor_scalar(mu, ps[0:1, 0], 1.0 / D, None, OP.mult)
            nc.gpsimd.tensor_tensor(m2, mu, mu, OP.mult)
            nc.vector.scalar_tensor_tensor(var, ps[0:1, 1], 1.0 / D, m2,
                                           OP.mult, OP.subtract)
            std = st[:, 1]  # reuse m2 slot
            nc.scalar.activation(std, var, AF.Sqrt, bias=eps1[:])
            rsf = st[:, 2]  # reuse var slot
            nc.vector.reciprocal_approx_fast(rsf, std)
            rssh = pool.tile([1, 2, RB], F32R, tag=f"rs{tag}", bufs=rs_bufs)
            with nc.allow_low_precision(reason="f32r matmul operand"):
                nc.gpsimd.tensor_copy(rssh[:, 0], rsf)
                # row 1: plain mean (the rank-1 mean-removal matmuls fold
                # the rs factor via the rs broadcast that follows)
                nc.gpsimd.tensor_copy(rssh[:, 1], mu)
            return rssh

        # attention SBUF pool + history loads live across stage A so the
        # first blocks' DMA happens under the projection phase
        hstack = ExitStack()
        hpool = hstack.enter_context(tc.tile_pool(name="attn_sb", bufs=1))
        st1 = {}  # blk -> (hf, hrt)

        def s_load(blk):
            hf = hpool.tile([P, DC, HCOL], FP8, tag="hf", bufs=6)
            nc.sync.dma_start(hf[:], hfmd[:, :, blk * HCOL :
                                            (blk + 1) * HCOL])
            hrt = hpool.tile([P, D2C, D], FP8, tag="hr", bufs=8)
            nc.sync.dma_start(hrt[:],
                              hrmd[:, blk * D2C : (blk + 1) * D2C, :])
            st1[blk] = (hf, hrt)

        # ================= Stage A: LN1 + Q2 projection =================
        with ExitStack() as ctx:
            apool = ctx.enter_context(tc.tile_pool(name="stage_a", bufs=2))
            stats_ps = ctx.enter_context(
                tc.tile_pool(name="a_stats", bufs=2, space="PSUM"))
            bc_ps = ctx.enter_context(
                tc.tile_pool(name="a_bcast", bufs=2, space="PSUM"))
            mm_ps = ctx.enter_context(
                tc.tile_pool(name="a_mm", bufs=2, space="PSUM"))

            astate = {}

            def a_front(rb):
                rsl = slice(rb * RB, (rb + 1) * RB)
                nc.sync.dma_start(xq8_t[:, :, rsl], xq8d[:, :, rsl])
                sq8 = apool.tile([P, DC, RB], FP8, tag="sq", bufs=2)
                astate[rb] = ln_stats(apool, stats_ps, xq8_t[:, :, rsl],
                                      sq8, "a")

            def a_back(rb):
                rsl = slice(rb * RB, (rb + 1) * RB)
                rssh = astate.pop(rb)
                ps_rsb = bc_ps.tile([P, RB], F32, tag="bc", bufs=2)
                nc.tensor.matmul(ps_rsb[:], onesA_t[:], rssh[:, 0],
                                 start=True, stop=True)
                rsb = apool.tile([P, RB], F32, tag="rsb", bufs=2)
                nc.scalar.copy(rsb[:], ps_rsb[:])
                for oc in range(DC):
                    ps_y = mm_ps.tile([P, RB], F32, tag="mm", bufs=2)
                    for pr in range(2):
                        nc.tensor.matmul(
                            ps_y[:], wa_t[:, 2 * pr : 2 * pr + 2,
                                          oc * P : (oc + 1) * P],
                            xq8_t[:, 2 * pr : 2 * pr + 2, rsl],
                            start=pr == 0, stop=False, perf_mode=DR)
                    # mean removal as a rank-1 accumulation: -S_A*cq (x) mu
                    nc.tensor.matmul(ps_y[:],
                                     cqn_t[:, oc * P : (oc + 1) * P],
                                     rssh[:, 1], start=False, stop=True)
                    t = apool.tile([P, RB], F32, tag="t", bufs=2)
                    nc.vector.tensor_tensor(t[:], ps_y[:], rsb[:], OP.mult)
                    with nc.allow_low_precision(reason="fp8 activation"):
                        nc.scalar.activation(q2_t[:, oc, rsl], t[:],
                                             AF.Identity,
                                             bias=dq_t[:, oc : oc + 1])

            a_front(0)
            a_front(1)
            load_late_consts()
            s_load(0)
            s_load(1)
            for rb in range(N_RB):
                if rb + 2 < N_RB:
                    a_front(rb + 2)
                a_back(rb)
                s_load(2 + rb)

        # ================= Stage B/C: attention =================
        # Software-pipelined; all transposes run on the PE in fp8 so the
        # tensor engine stays dense (HAM warm) and no DMA xbar is needed.
        # The decay/validity mask is a k=17 fp16 matmul accumulated into the
        # scores PSUM; exp's bias adds the -30 invalid offset and accum_out
        # yields softmax denominators.
        with ExitStack() as ctx:
            sc_ps = ctx.enter_context(
                tc.tile_pool(name="scores", bufs=2, space="PSUM"))
            ah_ps = ctx.enter_context(
                tc.tile_pool(name="attend", bufs=2, space="PSUM"))
            tr_ps = ctx.enter_context(
                tc.tile_pool(name="attn_tr", bufs=2, space="PSUM"))

            st2 = {}  # blk -> (am, den)
            st3 = {}  # blk -> amT8
            st4 = {}  # blk -> (atb8, r0)

            def s_scores(blk):
                hf, _ = st1[blk]
                r0 = blk * P
                ps_sc = sc_ps.tile([P, HCOL], F32, tag="sc", bufs=2)
                for nb in range(2):
                    for pr in range(2):
                        nc.tensor.matmul(
                            ps_sc[:, nb * RB : (nb + 1) * RB],
                            q2_t[:, 2 * pr : 2 * pr + 2, r0 : r0 + P],
                            hf[:, 2 * pr : 2 * pr + 2, nb * RB : (nb + 1) * RB],
                            start=pr == 0, stop=pr == 1, perf_mode=DR)
                nc.vector.tensor_tensor(ps_sc[:], ps_sc[:], mask_t[:], OP.add)
                am = hpool.tile([P, HCOL], FP8, tag="am", bufs=4)
                den = hpool.tile([P, 2], F32, tag="den", bufs=5)
                with nc.allow_low_precision(reason="fp8 attn probs"):
                    nc.scalar.activation(am[:], ps_sc[:], AF.Exp,
                                         scale=1.0 / S_Q,
                                         accum_out=den[:, 0:1])
                nc.vector.reciprocal_approx_fast(den[:, 1:2], den[:, 0:1])
                st2[blk] = (am, den)

            def s_trans(blk):
                am, _ = st2[blk]
                # fp8 PE-transpose writes with an element step of 2 in PSUM
                ps_t = tr_ps.tile([P, D2C, P, 2], FP8, tag="tr", bufs=2)
                for ch in range(D2C):
                    nc.tensor.transpose(ps_t[:, ch, :, 0],
                                        am[:, ch * P : (ch + 1) * P], id8_t[:])
                amT8 = hpool.tile([P, D2C, P], FP8, tag="amT8", bufs=3)
                with nc.allow_low_precision(reason="fp8 attn probs"):
                    nc.scalar.copy(amT8[:], ps_t[:, :, :, 0])
                st3[blk] = amT8

            def s_attend(blk):
                _, hrt = st1.pop(blk)
                _, den = st2.pop(blk)
                amT8 = st3.pop(blk)
                ps_ah = ah_ps.tile([P, D], F32, tag="ah", bufs=2)
                for pr in range(4):
                    nc.tensor.matmul(ps_ah[:], amT8[:, 2 * pr : 2 * pr + 2],
                                     hrt[:, 2 * pr : 2 * pr + 2],
                                     start=pr == 0, stop=pr == 3, perf_mode=DR)
                atb8 = hpool.tile([P, D], FP8, tag="atb8", bufs=3)
                with nc.allow_low_precision(reason="fp8 attended"):
                    nc.vector.tensor_scalar_mul(atb8[:], ps_ah[:], den[:, 1:2])
                st4[blk] = atb8

            def s_post(blk):
                atb8 = st4.pop(blk)
                r0 = blk * P
                ps_t = tr_ps.tile([P, D2C, P, 2], FP8, tag="tr", bufs=2)
                for ec in range(DC):
                    nc.tensor.transpose(ps_t[:, ec, :, 0],
                                        atb8[:, ec * P : (ec + 1) * P],
                                        id8_t[:])
                nc.vector.tensor_copy(att8_t[:, :, r0 : r0 + P],
                                      ps_t[:, :DC, :, 0])

            for i in range(N_BLK + 4):
                if 6 <= i + 2 < N_BLK:
                    s_load(i + 2)
                if 2 <= i < N_BLK + 2:
                    s_trans(i - 2)
                if 3 <= i < N_BLK + 3:
                    s_attend(i - 3)
                if i >= 4:
                    s_post(i - 4)
                if i < N_BLK:
                    s_scores(i)

        hstack.close()

        # ================= Stage D/E: Wov + LN2 + FFN =================
        # Three pipeline stages per rb: D (Wov projection + residual + LN2
        # stats, exact f32), E1 (LN2 apply + FFN1/gelu), E2 (FFN2 + out).
        with ExitStack() as ctx:
            epool = ctx.enter_context(tc.tile_pool(name="stage_e", bufs=2))
            stats_ps = ctx.enter_context(
                tc.tile_pool(name="e_stats", bufs=1, space="PSUM"))
            bc_ps = ctx.enter_context(
                tc.tile_pool(name="e_bcast", bufs=1, space="PSUM"))
            mm_ps = ctx.enter_context(
                tc.tile_pool(name="e_mm", bufs=3, space="PSUM"))

            onesr = const.tile([P, 1], F32R)
            nc.sync.dma_start(onesr[:], onesrd[:])

            dstate = {}
            estate = {}
            fstate = {}

            def e_projd(rb):
                rsl = slice(rb * RB, (rb + 1) * RB)
                x2 = epool.tile([P, DC, RB], F32R, tag="x2", bufs=4)
                for oc in range(DC):
                    ps_w = mm_ps.tile([P, RB], F32, tag="mm", bufs=3)
                    for pr in range(2):
                        nc.tensor.matmul(
                            ps_w[:], wov_t[:, 2 * pr : 2 * pr + 2,
                                           oc * P : (oc + 1) * P],
                            att8_t[:, 2 * pr : 2 * pr + 2, rsl],
                            start=pr == 0, stop=pr == 1, perf_mode=DR)
                    with nc.allow_low_precision(reason="f32r residual"):
                        nc.vector.scalar_tensor_tensor(
                            x2[:, oc], ps_w[:], 1.0 / S_O, xb16_t[:, oc, rsl],
                            OP.mult, OP.add)
                # exact f32 LN stats via f32r ones-matmuls on x2
                sqf = epool.tile([P, DC, RB], F32R, tag="sqf", bufs=2)
                with nc.allow_low_precision(reason="f32r square"):
                    nc.scalar.square(sqf[:], x2[:])
                ps = stats_ps.tile([1, 2, RB], F32, tag="st", bufs=1)
                for dc in range(DC):
                    nc.tensor.matmul(ps[:, 0], onesr[:], x2[:, dc],
                                     start=dc == 0, stop=dc == DC - 1)
                for dc in range(DC):
                    nc.tensor.matmul(ps[:, 1], onesr[:], sqf[:, dc],
                                     start=dc == 0, stop=dc == DC - 1)
                st = epool.tile([1, 3, RB], F32, tag="ste", bufs=2)
                mu, m2, var = st[:, 0], st[:, 1], st[:, 2]
                nc.vector.tensor_copy(mu, ps[0:1, 0])
                nc.gpsimd.tensor_tensor(m2, mu, mu, OP.mult)
                nc.vector.tensor_tensor(var, ps[0:1, 1], m2, OP.subtract)
                std = st[:, 1]
                nc.scalar.activation(std, var, AF.Sqrt, bias=eps1[:])
                rsf = st[:, 2]
                nc.vector.reciprocal_approx_fast(rsf, std)
                rssh = epool.tile([1, 2, RB], F32R, tag="rse", bufs=3)
                with nc.allow_low_precision(reason="f32r matmul operand"):
                    nc.gpsimd.tensor_copy(rssh[:, 0], rsf)
                    nc.gpsimd.tensor_tensor(rssh[:, 1], mu, rsf, OP.mult)
                dstate[rb] = (x2, rssh)

            def e_ln2(rb):
                x2, rssh = dstate.pop(rb)
                # h2_hat = x2 * rs  (g2 folded into W1; mean term removed
                # inside FFN1's PSUM as a rank-1 matmul; be2 folded into b1)
                ps_rs = bc_ps.tile([P, RB], F32, tag="bc", bufs=2)
                nc.tensor.matmul(ps_rs[:], ones1_t[:], rssh[:, 0],
                                 start=True, stop=True)
                h28 = epool.tile([P, DC, RB], FP8, tag="h2", bufs=2)
                with nc.allow_low_precision(reason="fp8 LN2 out"):
                    for dc in range(DC):
                        nc.vector.tensor_tensor(h28[:, dc], x2[:, dc],
                                                ps_rs[:], OP.mult)
                estate[rb] = (x2, rssh, h28)

            def e_ffn1(rb):
                x2, rssh, h28 = estate.pop(rb)
                a18 = epool.tile([P, D2C, RB], FP8, tag="a1", bufs=2)
                for oc in range(D2C):
                    ps_f = mm_ps.tile([P, RB], F32, tag="mm", bufs=3)
                    for pr in range(2):
                        nc.tensor.matmul(
                            ps_f[:], w1_t[:, 2 * pr : 2 * pr + 2,
                                          oc * P : (oc + 1) * P],
                            h28[:, 2 * pr : 2 * pr + 2],
                            start=pr == 0, stop=False, perf_mode=DR)
                    # rank-1 mean removal: -S_1*(W1@g2) (x) (mu*rs)
                    nc.tensor.matmul(ps_f[:],
                                     cf1n_t[:, oc * P : (oc + 1) * P],
                                     rssh[:, 1], start=False, stop=True)
                    with nc.allow_low_precision(reason="fp8 gelu"):
                        nc.scalar.activation(a18[:, oc], ps_f[:], AF.Gelu,
                                             bias=b1c_t[:, oc : oc + 1],
                                             scale=1.0 / S_1)
                fstate[rb] = (x2, a18)

            def e_ffn2(rb):
                rsl = slice(rb * RB, (rb + 1) * RB)
                x2, a18 = fstate.pop(rb)
                ot = epool.tile([P, DC, RB], BF16, tag="ot", bufs=2)
                tf = epool.tile([P, RB], F32, tag="tf", bufs=3)
                for oc in range(DC):
                    ps_f = mm_ps.tile([P, RB], F32, tag="mm", bufs=3)
                    for pr in range(4):
                        nc.tensor.matmul(
                            ps_f[:], w2_t[:, 2 * pr : 2 * pr + 2,
                                          oc * P : (oc + 1) * P],
                            a18[:, 2 * pr : 2 * pr + 2],
                            start=pr == 0, stop=pr == 3, perf_mode=DR)
                    nc.scalar.activation(tf[:], ps_f[:], AF.Identity,
                                         bias=b2c_t[:, oc : oc + 1],
                                         scale=1.0 / S_2)
                    with nc.allow_low_precision(reason="bf16 output"):
                        nc.vector.tensor_tensor(ot[:, oc], tf[:], x2[:, oc],
                                                OP.add)
                nc.sync.dma_start(outd[:, :, rsl], ot[:])

            e_projd(0)
            e_projd(1)
            e_ln2(0)
            e_projd(2)
            e_ffn1(0)
            e_ln2(1)
            e_projd(3)
            e_ffn2(0)
            e_ffn1(1)
            e_ln2(2)
            e_ffn2(1)
            e_ffn1(2)
            e_ln2(3)
            e_ffn2(2)
            e_ffn1(3)
            e_ffn2(3)

    nc.compile()
    return nc


def _tile_fm(a, dt):
    """[Dred, N] feature-major -> [128, Dred//128, N] device tiling."""
    dred, n = a.shape
    return np.ascontiguousarray(
        a.reshape(dred // P, P, n).swapaxes(0, 1)).astype(dt)


def _vec_pc(v):
    """[n*128] vector -> [128, n] (partition, chunk) f32."""
    return np.ascontiguousarray(np.asarray(v, np.float64).reshape(-1, P).T
                                ).astype(np.float32)


def _make_weight_map(inputs):
    f64 = {k: np.asarray(v, np.float64) for k, v in inputs.items()}
    isd = 1.0 / math.sqrt(D)

    WQK = (f64["Wq"].T @ f64["Wk"]) * isd          # [f, e]
    WA = f64["g1"][:, None] * WQK
    cq = WA.sum(axis=0)                            # [e]
    dq = f64["be1"] @ WQK + f64["bq"] @ f64["Wk"] * isd
    WOV = f64["Wo"] @ f64["Wv"]                    # [d, f]
    bo_p = f64["bo"] + f64["Wo"] @ f64["bv"]

    wa8 = _tile_fm(WA * S_A, ml_dtypes.float8_e4m3)
    wov8 = _tile_fm(WOV.T * S_O, ml_dtypes.float8_e4m3)
    W1g = f64["W1"] * f64["g2"][None, :]          # g2 folded into W1
    w18 = _tile_fm(W1g.T * S_1, ml_dtypes.float8_e4m3)
    w28 = _tile_fm(f64["W2"].T * S_2, ml_dtypes.float8_e4m3)
    cf1n = (-S_1 * (f64["W1"] @ f64["g2"]))[None, :]
    b1f = f64["b1"] + f64["W1"] @ f64["be2"]      # be2 folded into b1

    # additive scores mask: S_Q*log(decay) on valid (t-matching) entries,
    # S_Q*(-30) on invalid ones (exp flushes those to 0)
    h = np.arange(H)
    logdecay = np.log(DECAY_RATE ** (H - 1 - h) + 1e-10)
    maskS = np.full((P, HCOL), NEG_BIG * S_Q, np.float64)
    for p_ in range(P):
        t = p_ // B
        maskS[p_, t * H : (t + 1) * H] = S_Q * logdecay
    ones8 = np.ones((P, 2, 16), ml_dtypes.float8_e4m3)

    return dict(
        wa8=wa8, wov8=wov8, w18=w18, w28=w28,
        maskS=maskS.astype(np.float32),
        ident8=np.eye(P, dtype=ml_dtypes.float8_e4m3),
        dq=_vec_pc(dq * S_Q),
        b1c=_vec_pc(b1f),
        b2c=_vec_pc(f64["b2"]),
        be2c=_vec_pc(f64["be2"]),
        onesA=np.full((1, P), S_Q / S_A, np.float32),
        cqn=(-cq * S_A)[None, :].astype(np.float32),
        ones1=np.full((1, P), 1.0, np.float32),
        cf1n=cf1n.astype(np.float32),
        ones8=ones8,
        onesr=np.full((P, 1), 1.0 / D, np.float32),
        _bo_p=bo_p,  # consumed by core_input_map, not a dram tensor
    )


def core_input_map(inputs, wmap, c):
    """Per-core input dict (core c owns positions [c*T_LOC, (c+1)*T_LOC))."""
    x = np.asarray(inputs["x"], np.float32)
    history = np.asarray(inputs["history"], np.float32)
    ts = slice(c * T_LOC, (c + 1) * T_LOC)
    xr = x[:, ts, :].transpose(1, 0, 2).reshape(R, D)      # r = t*B + b
    hr = history[ts].reshape(HR, D)
    m = {k: v for k, v in wmap.items() if not k.startswith("_")}
    m["xq8"] = _tile_fm(np.ascontiguousarray(xr.T), ml_dtypes.float8_e4m3)
    m["xb16"] = _tile_fm(np.ascontiguousarray((xr + wmap["_bo_p"]).T),
                         ml_dtypes.bfloat16)
    m["hfm8"] = _tile_fm(np.ascontiguousarray(hr.T), ml_dtypes.float8_e4m3)
    m["hrm8"] = np.ascontiguousarray(
        hr.reshape(HR // P, P, D).swapaxes(0, 1)).astype(ml_dtypes.float8_e4m3)
    return m


def unpack_out(ot):
    """[128, DC, R] bf16 device tile -> [B, T_LOC, D] f32."""
    full = np.asarray(ot, np.float32).swapaxes(0, 1).reshape(D, R).T
    return full.reshape(T_LOC, B, D).transpose(1, 0, 2)


def kernel(**inputs):
    if "nc" not in _cached:
        _cached["nc"] = _build_program()
    nc = _cached["nc"]

    wmap = _make_weight_map(inputs)
    in_maps = [core_input_map(inputs, wmap, c) for c in range(N_CORES)]

    res = run_bass_kernel_spmd(nc, in_maps, core_ids=list(range(N_CORES)))
    _last_result[0] = res

    out = np.empty((B, T, D), dtype=np.float32)
    for c in range(N_CORES):
        ts = slice(c * T_LOC, (c + 1) * T_LOC)
        out[:, ts, :] = unpack_out(res.results[c]["outT"])
    return out


# revision 83
# speedup vs baseline: 1.0491x; 1.0259x over previous
"""Trainium2 Bass kernel for BandProcessorWithHistory (v2, algebraic rewrite).

Reference computation (full inputs):
    xn = LN(x, g1, be1); Q = xn@Wq.T + bq
    K = history@Wk.T + bk; V = history@Wv.T + bv          # [T,H,D]
    scores = einsum('btd,thd->bth', Q, K)/sqrt(D) + log(decay + 1e-10)
    attn = softmax(scores, -1); attended = einsum('bth,thd->btd', attn, V)
    x2 = x + attended@Wo.T + bo
    out = x2 + gelu(LN(x2,g2,be2)@W1.T + b1)@W2.T + b2

Key algebraic rewrite (removes the dominant K/V projections, 8x less work):
    scores  = Q.(H Wk^T)^T = (xn @ (Wq^T Wk)/sqrt(D)) . H^T        (+ bq@Wk;
              bk drops: softmax is invariant to a per-row constant)
    attended@Wo^T = (attn @ H) @ (Wo Wv)^T + Wo@bv   (rows of attn sum to 1)
so only [2048 x 512 x 512]-shaped projections remain per core, contracting
with raw history directly.  The per-position decay bias + block-diagonal
validity mask are folded into one additive f32 constant added to the scores
PSUM before exp; exp's accum_out produces softmax denominators for free.

Sharding: T (sequence) axis split over 8 cores (256 positions each);
embarrassingly parallel.  Activations feature-major [d on partitions, rows
on free dim], rows r = t_local*B + b.

Precision: matmuls run fp8e4(e4m3) with DoubleRow perf mode (2 k-chunks of
128 packed per instruction, ~1.5-2x PE throughput); weight matrices are
pre-scaled on the host into the fp8 normal range and descaled via activation
/vector-op scale constants.  LN statistics come from ones-matmuls (fp8 for
LN1 on the host-quantized x; exact f32r for LN2 on the on-chip x2).  The
per-row LN mean removal is algebraic: a k=1 rank-1 f32r matmul accumulated
into the downstream projection PSUM (-cq (x) mu for Q2, -(W1@g2) (x) mu*rs
for FFN1), with g2/be2 folded into W1/b1 on the host, so the LN apply is a
single broadcast-multiply.  Residual x rides in bf16 with bo' pre-added on
host; output is bf16.

All stages are software-pipelined 2-4 blocks deep so every engine queue
(in-order!) always holds ready work; attention transposes run on the PE in
fp8 (keeping the HAM clock-gate warm) with strided (step-2) PSUM outputs.
"""

import math
import os
from contextlib import ExitStack

import numpy as np
import ml_dtypes

import concourse.bacc as bacc
import concourse.bass as bass
import concourse.mybir as mybir
import concourse.tile as tile
from concourse.bass_utils import run_bass_kernel_spmd

F32 = mybir.dt.float32
F32R = mybir.dt.float32r
BF16 = mybir.dt.bfloat16
FP8 = mybir.dt.float8e4
DR = mybir.MatmulPerfMode.DoubleRow

B, T, H, D = 8, 2048, 64, 512
N_CORES = 8
T_LOC = T // N_CORES          # 256 positions per core
R = B * T_LOC                 # 2048 activation rows per core (r = t*B + b)
HR = T_LOC * H                # 16384 history rows per core
P = 128
DC = D // P                   # 4 chunks of the model dim
D2 = 2 * D
D2C = D2 // P                 # 8 chunks
BLK_T = 16                    # positions per attention block
N_BLK = T_LOC // BLK_T        # 16 blocks
HCOL = BLK_T * H              # 1024 history cols per block
RB = 512                      # r-columns per projection block
N_RB = R // RB                # 4
DECAY_RATE = 0.95
LN_EPS = 1e-5

# fp8 weight pre-scales (descaled on-chip via activation/vector constants)
S_A = 4096.0                  # LN1-folded Wq^T.Wk weight
S_Q = 256.0                   # Q2 activation scale (descaled inside exp)
S_O = 512.0                   # Wo.Wv weight
S_1 = 128.0                   # W1
S_2 = 128.0                   # W2
NEG_BIG = -30.0               # additive mask for invalid score entries

_last_result = [None]
_cached = {}

AF = mybir.ActivationFunctionType
OP = mybir.AluOpType


def _build_program():
    nc = bacc.Bacc("TRN2", target_bir_lowering=False, debug=False)

    xq8d = nc.dram_tensor("xq8", [P, DC, R], FP8, kind="ExternalInput")
    xb16d = nc.dram_tensor("xb16", [P, DC, R], BF16, kind="ExternalInput")
    hfmd = nc.dram_tensor("hfm8", [P, DC, HR], FP8, kind="ExternalInput")
    hrmd = nc.dram_tensor("hrm8", [P, HR // P, D], FP8, kind="ExternalInput")
    wad = nc.dram_tensor("wa8", [P, DC, D], FP8, kind="ExternalInput")
    wovd = nc.dram_tensor("wov8", [P, DC, D], FP8, kind="ExternalInput")
    w1d = nc.dram_tensor("w18", [P, DC, D2], FP8, kind="ExternalInput")
    w2d = nc.dram_tensor("w28", [P, D2C, D], FP8, kind="ExternalInput")
    maskd = nc.dram_tensor("maskS", [P, HCOL], F32, kind="ExternalInput")
    id8d = nc.dram_tensor("ident8", [P, P], FP8, kind="ExternalInput")
    dqd = nc.dram_tensor("dq", [P, DC], F32, kind="ExternalInput")
    b1cd = nc.dram_tensor("b1c", [P, D2C], F32, kind="ExternalInput")
    b2cd = nc.dram_tensor("b2c", [P, DC], F32, kind="ExternalInput")
    be2d = nc.dram_tensor("be2c", [P, DC], F32, kind="ExternalInput")
    onesAd = nc.dram_tensor("onesA", [1, P], F32R, kind="ExternalInput")
    cqnd = nc.dram_tensor("cqn", [1, D], F32R, kind="ExternalInput")
    ones1d = nc.dram_tensor("ones1", [1, P], F32R, kind="ExternalInput")
    cf1nd = nc.dram_tensor("cf1n", [1, D2], F32R, kind="ExternalInput")
    ones8d = nc.dram_tensor("ones8", [P, 2, 16], FP8, kind="ExternalInput")
    onesrd = nc.dram_tensor("onesr", [P, 1], F32R, kind="ExternalInput")
    outd = nc.dram_tensor("outT", [P, DC, R], BF16, kind="ExternalOutput")

    with tile.TileContext(nc) as tc, ExitStack() as top:
        const = top.enter_context(tc.tile_pool(name="const", bufs=1))
        pers = top.enter_context(tc.tile_pool(name="pers", bufs=1))

        # ---- constants resident for the whole kernel ----
        wa_t = const.tile([P, DC, D], FP8)
        ones8_t = const.tile([P, 2, 16], FP8)
        eps1 = const.tile([1, 1], F32)
        nc.vector.memset(eps1[:], LN_EPS)
        onesA_t = const.tile([1, P], F32R)
        cqn_t = const.tile([1, D], F32R)
        dq_t = const.tile([P, DC], F32)
        mask_t = const.tile([P, HCOL], F32)
        id8_t = const.tile([P, P], FP8)
        wov_t = const.tile([P, DC, D], FP8)
        w1_t = const.tile([P, DC, D2], FP8)
        w2_t = const.tile([P, D2C, D], FP8)
        ones1_t = const.tile([1, P], F32R)
        cf1n_t = const.tile([1, D2], F32R)
        b1c_t = const.tile([P, D2C], F32)
        b2c_t = const.tile([P, DC], F32)

        nc.sync.dma_start(wa_t[:], wad[:])
        nc.sync.dma_start(ones8_t[:], ones8d[:])
        nc.sync.dma_start(onesA_t[:], onesAd[:])
        nc.sync.dma_start(cqn_t[:], cqnd[:])
        nc.sync.dma_start(dq_t[:], dqd[:])

        def load_late_consts():
            nc.sync.dma_start(xb16_t[:], xb16d[:])
            nc.sync.dma_start(mask_t[:], maskd[:])
            nc.sync.dma_start(id8_t[:], id8d[:])
            nc.sync.dma_start(wov_t[:], wovd[:])
            nc.sync.dma_start(w1_t[:], w1d[:])
            nc.sync.dma_start(w2_t[:], w2d[:])
            nc.sync.dma_start(ones1_t[:], ones1d[:])
            nc.sync.dma_start(cf1n_t[:], cf1nd[:])
            nc.sync.dma_start(b1c_t[:], b1cd[:])
            nc.sync.dma_start(b2c_t[:], b2cd[:])

        # ---- per-core resident activations ----
        xq8_t = pers.tile([P, DC, R], FP8)
        xb16_t = pers.tile([P, DC, R], BF16)
        q2_t = pers.tile([P, DC, R], FP8)      # scaled by S_Q
        att8_t = pers.tile([P, DC, R], FP8)    # attended, feature-major



        def ln_stats(pool, stats_ps, src8, sq8, tag, bufs=2, rs_bufs=3):
            """LN stats of one [128, DC, RB] fp8 chunk -> (rs, sh) f32r rows.

            src8: fp8 source; sq8: same-shape scratch for squares.
            """
            with nc.allow_low_precision(reason="fp8 square"):
                nc.vector.tensor_tensor(sq8[:], src8, src8, OP.mult)
            ps = stats_ps.tile([16, 2, RB], F32, tag="st", bufs=bufs)
            for pr in range(2):
                nc.tensor.matmul(ps[:, 0], ones8_t[:],
                                 src8[:, 2 * pr : 2 * pr + 2],
                                 start=pr == 0, stop=pr == 1, perf_mode=DR)
            for pr in range(2):
                nc.tensor.matmul(ps[:, 1], ones8_t[:],
                                 sq8[:, 2 * pr : 2 * pr + 2],
                                 start=pr == 0, stop=pr == 1, perf_mode=DR)
            st = pool.tile([1, 3, RB], F32, tag=f"st{tag}", bufs=3)
            mu, m2, var = st[:, 0], st[:, 1], st[:, 2]
            nc.vector.tensor_scalar(mu, ps[0:1, 0], 1.0 / D, None, OP.mult)
            nc.gpsimd.tensor_tensor(m2, mu, mu, OP.mult)
            nc.vector.scalar_tensor_tensor(var, ps[0:1, 1], 1.0 / D, m2,
                                           OP.mult, OP.subtract)
            std = st[:, 1]  # reuse m2 slot
            nc.scalar.activation(std, var, AF.Sqrt, bias=eps1[:])
            rsf = st[:, 2]  # reuse var slot
            nc.vector.reciprocal_approx_fast(rsf, std)
            rssh = pool.tile([1, 2, RB], F32R, tag=f"rs{tag}", bufs=rs_bufs)
            with nc.allow_low_precision(reason="f32r matmul operand"):
                nc.gpsimd.tensor_copy(rssh[:, 0], rsf)
                # row 1: plain mean (the rank-1 mean-removal matmuls fold
                # the rs factor via the rs broadcast that follows)
                nc.gpsimd.tensor_copy(rssh[:, 1], mu)
            return rssh

        # attention SBUF pool + history loads live across stage A so the
        # first blocks' DMA happens under the projection phase
        hstack = ExitStack()
        hpool = hstack.enter_context(tc.tile_pool(name="attn_sb", bufs=1))
        st1 = {}  # blk -> (hf, hrt)

        def s_load(blk):
            hf = hpool.tile([P, DC, HCOL], FP8, tag="hf", bufs=6)
            nc.sync.dma_start(hf[:], hfmd[:, :, blk * HCOL :
                                            (blk + 1) * HCOL])
            hrt = hpool.tile([P, D2C, D], FP8, tag="hr", bufs=8)
            nc.sync.dma_start(hrt[:],
                              hrmd[:, blk * D2C : (blk + 1) * D2C, :])
            st1[blk] = (hf, hrt)

        # ================= Stage A: LN1 + Q2 projection =================
        with ExitStack() as ctx:
            apool = ctx.enter_context(tc.tile_pool(name="stage_a", bufs=2))
            stats_ps = ctx.enter_context(
                tc.tile_pool(name="a_stats", bufs=2, space="PSUM"))
            bc_ps = ctx.enter_context(
                tc.tile_pool(name="a_bcast", bufs=2, space="PSUM"))
            mm_ps = ctx.enter_context(
                tc.tile_pool(name="a_mm", bufs=2, space="PSUM"))

            astate = {}

            def a_front(rb):
                rsl = slice(rb * RB, (rb + 1) * RB)
                nc.sync.dma_start(xq8_t[:, :, rsl], xq8d[:, :, rsl])
                sq8 = apool.tile([P, DC, RB], FP8, tag="sq", bufs=2)
                astate[rb] = ln_stats(apool, stats_ps, xq8_t[:, :, rsl],
                                      sq8, "a")

            def a_back(rb):
                rsl = slice(rb * RB, (rb + 1) * RB)
                rssh = astate.pop(rb)
                ps_rsb = bc_ps.tile([P, RB], F32, tag="bc", bufs=2)
                nc.tensor.matmul(ps_rsb[:], onesA_t[:], rssh[:, 0],
                                 start=True, stop=True)
                rsb = apool.tile([P, RB], F32, tag="rsb", bufs=2)
                nc.scalar.copy(rsb[:], ps_rsb[:])
                for oc in range(DC):
                    ps_y = mm_ps.tile([P, RB], F32, tag="mm", bufs=2)
                    for pr in range(2):
                        nc.tensor.matmul(
                            ps_y[:], wa_t[:, 2 * pr : 2 * pr + 2,
                                          oc * P : (oc + 1) * P],
                            xq8_t[:, 2 * pr : 2 * pr + 2, rsl],
                            start=pr == 0, stop=False, perf_mode=DR)
                    # mean removal as a rank-1 accumulation: -S_A*cq (x) mu
                    nc.tensor.matmul(ps_y[:],
                                     cqn_t[:, oc * P : (oc + 1) * P],
                                     rssh[:, 1], start=False, stop=True)
                    t = apool.tile([P, RB], F32, tag="t", bufs=2)
                    nc.vector.tensor_tensor(t[:], ps_y[:], rsb[:], OP.mult)
                    with nc.allow_low_precision(reason="fp8 activation"):
                        nc.scalar.activation(q2_t[:, oc, rsl], t[:],
                                             AF.Identity,
                                             bias=dq_t[:, oc : oc + 1])

            a_front(0)
            a_front(1)
            load_late_consts()
            s_load(0)
            s_load(1)
            for rb in range(N_RB):
                if rb + 2 < N_RB:
                    a_front(rb + 2)
                a_back(rb)
                s_load(2 + rb)

        # ================= Stage B/C: attention =================
        # Software-pipelined; all transposes run on the PE in fp8 so the
        # tensor engine stays dense (HAM warm) and no DMA xbar is needed.
        # The decay/validity mask is a k=17 fp16 matmul accumulated into the
        # scores PSUM; exp's bias adds the -30 invalid offset and accum_out
        # yields softmax denominators.
        with ExitStack() as ctx:
            sc_ps = ctx.enter_context(
                tc.tile_pool(name="scores", bufs=2, space="PSUM"))
            ah_ps = ctx.enter_context(
                tc.tile_pool(name="attend", bufs=2, space="PSUM"))
            tr_ps = ctx.enter_context(
                tc.tile_pool(name="attn_tr", bufs=2, space="PSUM"))

            st2 = {}  # blk -> (am, den)
            st3 = {}  # blk -> amT8
            st4 = {}  # blk -> (atb8, r0)

            def s_scores(blk):
                hf, _ = st1[blk]
                r0 = blk * P
                ps_sc = sc_ps.tile([P, HCOL], F32, tag="sc", bufs=2)
                for nb in range(2):
                    for pr in range(2):
                        nc.tensor.matmul(
                            ps_sc[:, nb * RB : (nb + 1) * RB],
                            q2_t[:, 2 * pr : 2 * pr + 2, r0 : r0 + P],
                            hf[:, 2 * pr : 2 * pr + 2, nb * RB : (nb + 1) * RB],
                            start=pr == 0, stop=pr == 1, perf_mode=DR)
                nc.vector.tensor_tensor(ps_sc[:], ps_sc[:], mask_t[:], OP.add)
                am = hpool.tile([P, HCOL], FP8, tag="am", bufs=4)
                den = hpool.tile([P, 2], F32, tag="den", bufs=5)
                with nc.allow_low_precision(reason="fp8 attn probs"):
                    nc.scalar.activation(am[:], ps_sc[:], AF.Exp,
                                         scale=1.0 / S_Q,
                                         accum_out=den[:, 0:1])
                nc.vector.reciprocal_approx_fast(den[:, 1:2], den[:, 0:1])
                st2[blk] = (am, den)

            def s_trans(blk):
                am, _ = st2[blk]
                # fp8 PE-transpose writes with an element step of 2 in PSUM
                ps_t = tr_ps.tile([P, D2C, P, 2], FP8, tag="tr", bufs=2)
                for ch in range(D2C):
                    nc.tensor.transpose(ps_t[:, ch, :, 0],
                                        am[:, ch * P : (ch + 1) * P], id8_t[:])
                amT8 = hpool.tile([P, D2C, P], FP8, tag="amT8", bufs=3)
                with nc.allow_low_precision(reason="fp8 attn probs"):
                    nc.scalar.copy(amT8[:], ps_t[:, :, :, 0])
                st3[blk] = amT8

            def s_attend(blk):
                _, hrt = st1.pop(blk)
                _, den = st2.pop(blk)
                amT8 = st3.pop(blk)
                ps_ah = ah_ps.tile([P, D], F32, tag="ah", bufs=2)
                for pr in range(4):
                    nc.tensor.matmul(ps_ah[:], amT8[:, 2 * pr : 2 * pr + 2],
                                     hrt[:, 2 * pr : 2 * pr + 2],
                                     start=pr == 0, stop=pr == 3, perf_mode=DR)
                atb8 = hpool.tile([P, D], FP8, tag="atb8", bufs=3)
                with nc.allow_low_precision(reason="fp8 attended"):
                    nc.vector.tensor_scalar_mul(atb8[:], ps_ah[:], den[:, 1:2])
                st4[blk] = atb8

            def s_post(blk):
                atb8 = st4.pop(blk)
                r0 = blk * P
                ps_t = tr_ps.tile([P, D2C, P, 2], FP8, tag="tr", bufs=2)
                for ec in range(DC):
                    nc.tensor.transpose(ps_t[:, ec, :, 0],
                                        atb8[:, ec * P : (ec + 1) * P],
                                        id8_t[:])
                nc.vector.tensor_copy(att8_t[:, :, r0 : r0 + P],
                                      ps_t[:, :DC, :, 0])

            for i in range(N_BLK + 4):
                if 6 <= i + 2 < N_BLK:
                    s_load(i + 2)
                if 2 <= i < N_BLK + 2:
                    s_trans(i - 2)
                if 3 <= i < N_BLK + 3:
                    s_attend(i - 3)
                if i >= 4:
                    s_post(i - 4)
                if i < N_BLK:
                    s_scores(i)

        hstack.close()

        # ================= Stage D/E: Wov + LN2 + FFN =================
        # Three pipeline stages per rb: D (Wov projection + residual + LN2
        # stats, exact f32), E1 (LN2 apply + FFN1/gelu), E2 (FFN2 + out).
        with ExitStack() as ctx:
            epool = ctx.enter_context(tc.tile_pool(name="stage_e", bufs=2))
            stats_ps = ctx.enter_context(
                tc.tile_pool(name="e_stats", bufs=1, space="PSUM"))
            bc_ps = ctx.enter_context(
                tc.tile_pool(name="e_bcast", bufs=1, space="PSUM"))
            mm_ps = ctx.enter_context(
                tc.tile_pool(name="e_mm", bufs=3, space="PSUM"))

            onesr = const.tile([P, 1], F32R)
            nc.sync.dma_start(onesr[:], onesrd[:])

            dstate = {}
            estate = {}
            fstate = {}

            def e_projd(rb):
                rsl = slice(rb * RB, (rb + 1) * RB)
                x2 = epool.tile([P, DC, RB], F32R, tag="x2", bufs=4)
                for oc in range(DC):
                    ps_w = mm_ps.tile([P, RB], F32, tag="mm", bufs=4)
                    for pr in range(2):
                        nc.tensor.matmul(
                            ps_w[:], wov_t[:, 2 * pr : 2 * pr + 2,
                                           oc * P : (oc + 1) * P],
                            att8_t[:, 2 * pr : 2 * pr + 2, rsl],
                            start=pr == 0, stop=pr == 1, perf_mode=DR)
                    with nc.allow_low_precision(reason="f32r residual"):
                        nc.vector.scalar_tensor_tensor(
                            x2[:, oc], ps_w[:], 1.0 / S_O, xb16_t[:, oc, rsl],
                            OP.mult, OP.add)
                # exact f32 LN stats via f32r ones-matmuls on x2
                sqf = epool.tile([P, DC, RB], F32R, tag="sqf", bufs=2)
                with nc.allow_low_precision(reason="f32r square"):
                    nc.scalar.square(sqf[:], x2[:])
                ps = stats_ps.tile([1, 2, RB], F32, tag="st", bufs=1)
                for dc in range(DC):
                    nc.tensor.matmul(ps[:, 0], onesr[:], x2[:, dc],
                                     start=dc == 0, stop=dc == DC - 1)
                for dc in range(DC):
                    nc.tensor.matmul(ps[:, 1], onesr[:], sqf[:, dc],
                                     start=dc == 0, stop=dc == DC - 1)
                st = epool.tile([1, 3, RB], F32, tag="ste", bufs=2)
                mu, m2, var = st[:, 0], st[:, 1], st[:, 2]
                nc.vector.tensor_copy(mu, ps[0:1, 0])
                nc.gpsimd.tensor_tensor(m2, mu, mu, OP.mult)
                nc.vector.tensor_tensor(var, ps[0:1, 1], m2, OP.subtract)
                std = st[:, 1]
                nc.scalar.activation(std, var, AF.Sqrt, bias=eps1[:])
                rsf = st[:, 2]
                nc.vector.reciprocal_approx_fast(rsf, std)
                rssh = epool.tile([1, 2, RB], F32R, tag="rse", bufs=3)
                with nc.allow_low_precision(reason="f32r matmul operand"):
                    nc.gpsimd.tensor_copy(rssh[:, 0], rsf)
                    nc.gpsimd.tensor_tensor(rssh[:, 1], mu, rsf, OP.mult)
                dstate[rb] = (x2, rssh)

            def e_ln2(rb):
                x2, rssh = dstate.pop(rb)
                # h2_hat = x2 * rs  (g2 folded into W1; mean term removed
                # inside FFN1's PSUM as a rank-1 matmul; be2 folded into b1)
                ps_rs = bc_ps.tile([P, RB], F32, tag="bc", bufs=2)
                nc.tensor.matmul(ps_rs[:], ones1_t[:], rssh[:, 0],
                                 start=True, stop=True)
                h28 = epool.tile([P, DC, RB], FP8, tag="h2", bufs=2)
                with nc.allow_low_precision(reason="fp8 LN2 out"):
                    for dc in range(DC):
                        nc.vector.tensor_tensor(h28[:, dc], x2[:, dc],
                                                ps_rs[:], OP.mult)
                estate[rb] = (x2, rssh, h28)

            def e_ffn1(rb):
                x2, rssh, h28 = estate.pop(rb)
                a18 = epool.tile([P, D2C, RB], FP8, tag="a1", bufs=2)
                for oc in range(D2C):
                    ps_f = mm_ps.tile([P, RB], F32, tag="mm", bufs=4)
                    for pr in range(2):
                        nc.tensor.matmul(
                            ps_f[:], w1_t[:, 2 * pr : 2 * pr + 2,
                                          oc * P : (oc + 1) * P],
                            h28[:, 2 * pr : 2 * pr + 2],
                            start=pr == 0, stop=False, perf_mode=DR)
                    # rank-1 mean removal: -S_1*(W1@g2) (x) (mu*rs)
                    nc.tensor.matmul(ps_f[:],
                                     cf1n_t[:, oc * P : (oc + 1) * P],
                                     rssh[:, 1], start=False, stop=True)
                    with nc.allow_low_precision(reason="fp8 gelu"):
                        nc.scalar.activation(a18[:, oc], ps_f[:], AF.Gelu,
                                             bias=b1c_t[:, oc : oc + 1],
                                             scale=1.0 / S_1)
                fstate[rb] = (x2, a18)

            def e_ffn2(rb):
                rsl = slice(rb * RB, (rb + 1) * RB)
                x2, a18 = fstate.pop(rb)
                ot = epool.tile([P, DC, RB], BF16, tag="ot", bufs=2)
                tf = epool.tile([P, RB], F32, tag="tf", bufs=3)
                for oc in range(DC):
                    ps_f = mm_ps.tile([P, RB], F32, tag="mm", bufs=4)
                    for pr in range(4):
                        nc.tensor.matmul(
                            ps_f[:], w2_t[:, 2 * pr : 2 * pr + 2,
                                          oc * P : (oc + 1) * P],
                            a18[:, 2 * pr : 2 * pr + 2],
                            start=pr == 0, stop=pr == 3, perf_mode=DR)
                    nc.scalar.activation(tf[:], ps_f[:], AF.Identity,
                                         bias=b2c_t[:, oc : oc + 1],
                                         scale=1.0 / S_2)
                    with nc.allow_low_precision(reason="bf16 output"):
                        nc.gpsimd.tensor_tensor(ot[:, oc], tf[:], x2[:, oc],
                                                OP.add)
                nc.sync.dma_start(outd[:, :, rsl], ot[:])

            e_projd(0)
            e_projd(1)
            e_ln2(0)
            e_projd(2)
            e_ffn1(0)
            e_ln2(1)
            e_projd(3)
            e_ffn2(0)
            e_ffn1(1)
            e_ln2(2)
            e_ffn2(1)
            e_ffn1(2)
            e_ln2(3)
            e_ffn2(2)
            e_ffn1(3)
            e_ffn2(3)

    nc.compile()
    return nc


def _tile_fm(a, dt):
    """[Dred, N] feature-major -> [128, Dred//128, N] device tiling."""
    dred, n = a.shape
    return np.ascontiguousarray(
        a.reshape(dred // P, P, n).swapaxes(0, 1)).astype(dt)


def _vec_pc(v):
    """[n*128] vector -> [128, n] (partition, chunk) f32."""
    return np.ascontiguousarray(np.asarray(v, np.float64).reshape(-1, P).T
                                ).astype(np.float32)


def _make_weight_map(inputs):
    f64 = {k: np.asarray(v, np.float64) for k, v in inputs.items()}
    isd = 1.0 / math.sqrt(D)

    WQK = (f64["Wq"].T @ f64["Wk"]) * isd          # [f, e]
    WA = f64["g1"][:, None] * WQK
    cq = WA.sum(axis=0)                            # [e]
    dq = f64["be1"] @ WQK + f64["bq"] @ f64["Wk"] * isd
    WOV = f64["Wo"] @ f64["Wv"]                    # [d, f]
    bo_p = f64["bo"] + f64["Wo"] @ f64["bv"]

    wa8 = _tile_fm(WA * S_A, ml_dtypes.float8_e4m3)
    wov8 = _tile_fm(WOV.T * S_O, ml_dtypes.float8_e4m3)
    W1g = f64["W1"] * f64["g2"][None, :]          # g2 folded into W1
    w18 = _tile_fm(W1g.T * S_1, ml_dtypes.float8_e4m3)
    w28 = _tile_fm(f64["W2"].T * S_2, ml_dtypes.float8_e4m3)
    cf1n = (-S_1 * (f64["W1"] @ f64["g2"]))[None, :]
    b1f = f64["b1"] + f64["W1"] @ f64["be2"]      # be2 folded into b1

    # additive scores mask: S_Q*log(decay) on valid (t-matching) entries,
    # S_Q*(-30) on invalid ones (exp flushes those to 0)
    h = np.arange(H)
    logdecay = np.log(DECAY_RATE ** (H - 1 - h) + 1e-10)
    maskS = np.full((P, HCOL), NEG_BIG * S_Q, np.float64)
    for p_ in range(P):
        t = p_ // B
        maskS[p_, t * H : (t + 1) * H] = S_Q * logdecay
    ones8 = np.ones((P, 2, 16), ml_dtypes.float8_e4m3)

    return dict(
        wa8=wa8, wov8=wov8, w18=w18, w28=w28,
        maskS=maskS.astype(np.float32),
        ident8=np.eye(P, dtype=ml_dtypes.float8_e4m3),
        dq=_vec_pc(dq * S_Q),
        b1c=_vec_pc(b1f),
        b2c=_vec_pc(f64["b2"]),
        be2c=_vec_pc(f64["be2"]),
        onesA=np.full((1, P), S_Q / S_A, np.float32),
        cqn=(-cq * S_A)[None, :].astype(np.float32),
        ones1=np.full((1, P), 1.0, np.float32),
        cf1n=cf1n.astype(np.float32),
        ones8=ones8,
        onesr=np.full((P, 1), 1.0 / D, np.float32),
        _bo_p=bo_p,  # consumed by core_input_map, not a dram tensor
    )


def core_input_map(inputs, wmap, c):
    """Per-core input dict (core c owns positions [c*T_LOC, (c+1)*T_LOC))."""
    x = np.asarray(inputs["x"], np.float32)
    history = np.asarray(inputs["history"], np.float32)
    ts = slice(c * T_LOC, (c + 1) * T_LOC)
    xr = x[:, ts, :].transpose(1, 0, 2).reshape(R, D)      # r = t*B + b
    hr = history[ts].reshape(HR, D)
    m = {k: v for k, v in wmap.items() if not k.startswith("_")}
    m["xq8"] = _tile_fm(np.ascontiguousarray(xr.T), ml_dtypes.float8_e4m3)
    m["xb16"] = _tile_fm(np.ascontiguousarray((xr + wmap["_bo_p"]).T),
                         ml_dtypes.bfloat16)
    m["hfm8"] = _tile_fm(np.ascontiguousarray(hr.T), ml_dtypes.float8_e4m3)
    m["hrm8"] = np.ascontiguousarray(
        hr.reshape(HR // P, P, D).swapaxes(0, 1)).astype(ml_dtypes.float8_e4m3)
    return m


def unpack_out(ot):
    """[128, DC, R] bf16 device tile -> [B, T_LOC, D] f32."""
    full = np.asarray(ot, np.float32).swapaxes(0, 1).reshape(D, R).T
    return full.reshape(T_LOC, B, D).transpose(1, 0, 2)


def kernel(**inputs):
    if "nc" not in _cached:
        _cached["nc"] = _build_program()
    nc = _cached["nc"]

    wmap = _make_weight_map(inputs)
    in_maps = [core_input_map(inputs, wmap, c) for c in range(N_CORES)]

    res = run_bass_kernel_spmd(nc, in_maps, core_ids=list(range(N_CORES)))
    _last_result[0] = res

    out = np.empty((B, T, D), dtype=np.float32)
    for c in range(N_CORES):
        ts = slice(c * T_LOC, (c + 1) * T_LOC)
        out[:, ts, :] = unpack_out(res.results[c]["outT"])
    return out
